# revision 1
# baseline (speedup 1.0000x reference)
"""8-way sharded MultiHeadAttention for Trainium2 (Bass/Tile).

Problem: B=2, S=2048, H=1024, NH=16 heads of D=64.
  out = softmax((x@wq.T+bq) @ (x@wk.T+bk).T / sqrt(D)) @ (x@wv.T+bv),
  concat heads, @ wo.T + bo.

Sharding (Megatron-style tensor parallel over 8 NeuronCores):
  core c owns batch b = c//4 and the 4 heads 4*(c%4)..4*(c%4)+3
  (feature columns Ic = 256*(c%4) .. +256 of q/k/v).
  - column-parallel QKV projections (each core projects all tokens of its
    batch onto its 256 feature columns)
  - attention fully local per head
  - row-parallel output projection producing a partial [H, S] result;
    the 4 partials per batch are summed on the host (no device collective)

Per-core on-device layout notes:
  - everything is computed in [feature, token] layout; the host passes
    x[b].T and pre-transposed weight slices so no on-device transposes
    are needed.
  - scores are computed transposed ([key, query]) so the softmax sum folds
    into the attn@v matmul via a ones-augmented V ([v | 1]).
  - exp runs on the scalar engine straight out of PSUM with the 1/sqrt(D)
    scale fused into the activation. No max-subtraction: with this
    problem's distributions |scores/8| < ~4, exp is safe in fp32 (softmax
    is shift-invariant so the result matches the reference).
  - matmuls use the float32r feed mode (full-rate fp32 on the PE for
    moving dims >= 256 vs 4 cycles/row for plain fp32).
  - softmax normalization happens at PSUM eviction: 1/Z (DVE reciprocal)
    is broadcast across partitions by a K=1 matmul and fused into the
    eviction multiply.
"""

import sys

for _p in ("/opt/trn_rl_repo", "/root/.axon_site/_ro/trn_rl_repo"):
    if _p not in sys.path:
        sys.path.append(_p)

from contextlib import ExitStack

import numpy as np

import concourse.bass as bass
import concourse.mybir as mybir
import concourse.tile as tile
from concourse import bacc
from concourse.bass_utils import run_bass_kernel_spmd

F32 = mybir.dt.float32
F32R = mybir.dt.float32r
AF = mybir.ActivationFunctionType

P = 128
B = 2
S = 2048          # tokens
H = 1024          # hidden
KO = H // P       # 8 k-chunks for the QKV projections
MO = 2            # 256 local features / 128
HEADS = 4         # heads per core
D = 64
NKT = S // P      # 16 key chunks
HALF = 1024       # qt half width
NCORES = 8

# test.py can flip these before calling kernel()
TRACE = False
LAST_RESULT = {}


def _r(ap):
    return ap


def _normalize_evict(nc, sm, big, ctx_sb, ctx_ps, ones_sb, o, prow, half, r):
    """Softmax-normalize and evict one head-half from PSUM.

    The ctx PSUM slot gates the next head pair's attn@v (cxp bufs=2), so
    release it as fast as possible: 1/Z straight from the PSUM Z row, raw
    ctx copied to SBUF — two independent DVE ops — then the broadcast
    matmul and the normalize multiply run off the critical path. Odd heads
    (prow=64) write to partitions 64-127 of ctx_sb (partition-shifted DVE
    write)."""
    rst = sm.tile([P, HALF], F32R, tag="rst", name="rst")
    with nc.allow_low_precision(
        reason="1/Z in f32r: Z ~ O(S), plenty of headroom"
    ):
        nc.vector.reciprocal(rst[64:65, :], ctx_ps[64:65, :])
    traw = sm.tile([P, HALF], F32, tag="traw", name="traw")
    nc.vector.tensor_copy(traw[0:64, :], ctx_ps[0:64, :])
    bc = big.tile([P, HALF], F32, tag="ctx_ps", name="bc")
    for j in range(2):
        nc.tensor.matmul(
            bc[0:64, j * 512:(j + 1) * 512],
            lhsT=r(ones_sb[64:65, :]),
            rhs=r(rst[64:65, j * 512:(j + 1) * 512]),
            start=True, stop=True,
        )
    nc.vector.tensor_tensor(
        ctx_sb[prow:prow + 64, o, half * HALF:(half + 1) * HALF],
        traw[0:64, :],
        bc[0:64, :],
        mybir.AluOpType.mult,
    )


def build_mha_kernel(nc: bass.Bass):
    xT = nc.declare_dram_parameter("xT", [H, S], F32R, isOutput=False)
    wqT = nc.declare_dram_parameter("wqT", [H, 256], F32R, isOutput=False)
    wkT = nc.declare_dram_parameter("wkT", [H, 256], F32R, isOutput=False)
    wvT = nc.declare_dram_parameter("wvT", [H, 256], F32R, isOutput=False)
    bq2 = nc.declare_dram_parameter("bq2", [P, MO], F32, isOutput=False)
    bk2 = nc.declare_dram_parameter("bk2", [P, MO], F32, isOutput=False)
    bv2 = nc.declare_dram_parameter("bv2", [P, 256], F32, isOutput=False)
    woT = nc.declare_dram_parameter("woT", [256, H], F32R, isOutput=False)
    ones_d = nc.declare_dram_parameter("ones_d", [P, 64], F32R, isOutput=False)
    vones_d = nc.declare_dram_parameter("vones_d", [P, NKT, HEADS, 1], F32R,
                                        isOutput=False)
    poutT = nc.declare_dram_parameter("poutT", [H, S], F32, isOutput=True)

    xT_r = xT.rearrange("(o p) n -> p o n", p=P)        # [128, 8, 2048]
    wq_r = wqT.rearrange("(o p) m -> p o m", p=P)       # [128, 8, 256]
    wk_r = wkT.rearrange("(o p) m -> p o m", p=P)
    wv_r = wvT.rearrange("(o p) m -> p o m", p=P)
    wo_r = woT.rearrange("(o p) m -> p o m", p=P)       # [128, 2, 1024]
    pout_r = poutT.rearrange("(o p) n -> p o n", p=P)   # [128, 8, 2048]

    r = _r
    with tile.TileContext(nc) as tc, ExitStack() as ctx:
        xp = ctx.enter_context(tc.tile_pool(name="xp", bufs=1))
        wp = ctx.enter_context(tc.tile_pool(name="wp", bufs=1))
        qk = ctx.enter_context(tc.tile_pool(name="qk", bufs=1))
        vp = ctx.enter_context(tc.tile_pool(name="vp", bufs=1))
        pp = ctx.enter_context(tc.tile_pool(name="pp", bufs=2))
        cx = ctx.enter_context(tc.tile_pool(name="cx", bufs=1))
        sm = ctx.enter_context(tc.tile_pool(name="sm", bufs=2))
        ob = ctx.enter_context(tc.tile_pool(name="ob", bufs=5))
        big = ctx.enter_context(tc.tile_pool(name="big", bufs=2, space="PSUM"))
        cxp = ctx.enter_context(tc.tile_pool(name="cxp", bufs=2, space="PSUM"))

        # ---- load everything (chunked per k-slice: fewer DMA-queue sems
        # per consuming matmul, and lets compute start before the full
        # 8MB of x has landed) ----
        x_sb = xp.tile([P, KO, S], F32R)
        wq_sb = wp.tile([P, KO, 256], F32R, tag="wq")
        wk_sb = wp.tile([P, KO, 256], F32R, tag="wk")
        wv_sb = wp.tile([P, KO, 256], F32R, tag="wv")
        wo_sb = wp.tile([P, MO, H], F32R, tag="wo")
        # spread the big input loads across DGE queues so the 8MB of x
        # streams in parallel instead of serializing on one ring
        _eng = [nc.sync, nc.gpsimd, nc.scalar]
        for k in range(KO):
            _eng[k % 3].dma_start(x_sb[:, k, :], xT_r[:, k, :])
            _eng[(k + 1) % 3].dma_start(wq_sb[:, k, :], wq_r[:, k, :])
            _eng[(k + 2) % 3].dma_start(wk_sb[:, k, :], wk_r[:, k, :])
            _eng[k % 3].dma_start(wv_sb[:, k, :], wv_r[:, k, :])
        for k2 in range(MO):
            nc.sync.dma_start(wo_sb[:, k2, :], wo_r[:, k2, :])
        bq_sb = wp.tile([P, MO], F32, tag="bq")
        bk_sb = wp.tile([P, MO], F32, tag="bk")
        bv_sb = wp.tile([P, 256], F32, tag="bv")
        ones_sb = wp.tile([P, 64], F32R, tag="ones")
        nc.sync.dma_start(bq_sb[:], bq2[:])
        nc.sync.dma_start(bk_sb[:], bk2[:])
        nc.sync.dma_start(bv_sb[:], bv2[:])
        nc.sync.dma_start(ones_sb[:], ones_d[:])

        qT_sb = qk.tile([P, MO, S], F32R, tag="q")       # [feat, token]
        kT_sb = qk.tile([P, MO, S], F32R, tag="k")
        # v in [token, head, 65] layout: [v | 1] per head
        v_sb = vp.tile([P, NKT, HEADS, 65], F32R)

        # ones columns for the softmax-sum rows (independent of v evicts)
        nc.sync.dma_start(v_sb[:, :, :, 64:65], vones_d[:])
        ctx_sb = cx.tile([P, MO, S], F32R)

        def emit_qk(m):
            # q/k projections for feature chunk m: qT = wqT.T @ xT
            for w_sb, b_sb, dst in ((wq_sb, bq_sb, qT_sb),
                                    (wk_sb, bk_sb, kT_sb)):
                for tp2 in range(2):  # 1024-token chunks
                    ps = big.tile([P, HALF], F32, tag="big", name="ps")
                    for k in range(KO):
                        for j in range(2):
                            nc.tensor.matmul(
                                ps[:, j * 512:(j + 1) * 512],
                                lhsT=r(w_sb[:, k, m * P:(m + 1) * P]),
                                rhs=r(x_sb[:, k, tp2 * HALF + j * 512:
                                           tp2 * HALF + (j + 1) * 512]),
                                start=(k == 0), stop=(k == KO - 1),
                            )
                    nc.vector.tensor_tensor(
                        dst[:, m, tp2 * HALF:(tp2 + 1) * HALF],
                        ps[:],
                        b_sb[:, m:m + 1].to_broadcast((P, HALF)),
                        mybir.AluOpType.add,
                    )

        def emit_v():
            # v projection (all heads; rhs N=256 keeps fp32r at full rate)
            # in [token, feat] layout
            for tc4 in range(4):  # 4 chunks of 4*128 tokens
                ps = big.tile([P, 4, 256], F32, tag="big", name="ps")
                for ktl in range(4):
                    kt = tc4 * 4 + ktl
                    for k in range(KO):
                        nc.tensor.matmul(
                            ps[:, ktl, :],
                            lhsT=r(x_sb[:, k, kt * P:(kt + 1) * P]),
                            rhs=r(wv_sb[:, k, :]),
                            start=(k == 0), stop=(k == KO - 1),
                        )
                for ktl in range(4):
                    kt = tc4 * 4 + ktl
                    for h in range(HEADS):
                        nc.vector.tensor_tensor(
                            v_sb[:, kt, h, 0:64],
                            ps[:, ktl, h * 64:(h + 1) * 64],
                            bv_sb[:, h * 64:(h + 1) * 64],
                            mybir.AluOpType.add,
                        )

        def emit_attn(half, pair):
            # two heads interleaved per key-chunk: while ACT runs exp for
            # one head, the PE computes the other head's scores / attn@v
            ctx_t = [None, None]  # per head-in-pair
            for kt in range(NKT):
                for hi in range(2):
                    h = 2 * pair + hi
                    o, prow = h // 2, 64 * (h % 2)
                    qh = qT_sb[prow:prow + 64, o, :]
                    kh = kT_sb[prow:prow + 64, o, :]
                    sp = big.tile([P, HALF], F32, tag="big", name="sp")
                    for j in range(2):
                        nc.tensor.matmul(
                            sp[:, j * 512:(j + 1) * 512],
                            lhsT=r(kh[:, kt * P:(kt + 1) * P]),
                            rhs=r(qh[:, half * HALF + j * 512:
                                     half * HALF + (j + 1) * 512]),
                            start=True, stop=True,
                        )
                    pt = pp.tile([P, HALF], F32R)
                    nc.scalar.activation(pt[:], sp[:], AF.Exp, scale=0.125)
                    if kt == 0:
                        ctx_t[hi] = cxp.tile([P, HALF], F32, name="ctx_ps",
                                             tag="ctx_ps")
                    # out rows 0..64: ctx at 0-63, Z at 64 (all heads)
                    for j in range(2):
                        nc.tensor.matmul(
                            ctx_t[hi][0:65, j * 512:(j + 1) * 512],
                            lhsT=r(v_sb[:, kt, h, :]),
                            rhs=r(pt[:, j * 512:(j + 1) * 512]),
                            start=(kt == 0), stop=(kt == NKT - 1),
                        )
            for hi in range(2):
                h = 2 * pair + hi
                o, prow = h // 2, 64 * (h % 2)
                _normalize_evict(nc, sm, cxp, ctx_sb, ctx_t[hi], ones_sb,
                                 o, prow, half, r)

        def emit_outproj(tp):
            # out-proj for token half tp; deprioritized so it loses ties
            # against attention and only fills PE gaps
            with tc.high_priority(offset=-(10 ** 6)):
                for m in range(KO):
                    ps = big.tile([P, HALF], F32, tag="big", name="ps")
                    for k2 in range(MO):
                        for j in range(2):
                            nc.tensor.matmul(
                                ps[:, j * 512:(j + 1) * 512],
                                lhsT=r(wo_sb[:, k2, m * P:(m + 1) * P]),
                                rhs=r(ctx_sb[:, k2, tp * HALF + j * 512:
                                             tp * HALF + (j + 1) * 512]),
                                start=(k2 == 0), stop=(k2 == MO - 1),
                            )
                    ot = ob.tile([P, HALF], F32)
                    nc.vector.tensor_copy(ot[:], ps[:])
                    nc.sync.dma_start(
                        pout_r[:, m, tp * HALF:(tp + 1) * HALF], ot[:])

        # Emission order: projections first (PE-dense), then per token-half
        # attention with the half's out-projection interleaved so its
        # output DMA hides under the other half's attention. (Starting
        # pair-0 attention before the m=1 projections was tried and is
        # slower: early attention steals PE from the projections and
        # delays everything downstream.)
        emit_qk(0)
        emit_qk(1)
        emit_v()
        emit_attn(0, 0)
        emit_attn(0, 1)
        emit_outproj(0)
        emit_attn(1, 0)
        emit_attn(1, 1)
        emit_outproj(1)

    return nc


_NC_CACHE = []


def _get_nc():
    if not _NC_CACHE:
        nc = bacc.Bacc(
            "TRN2",
            target_bir_lowering=False,
            debug=False,
            enable_asserts=False,
            num_devices=NCORES,
        )
        build_mha_kernel(nc)
        nc.finalize()
        _NC_CACHE.append(nc)
    return _NC_CACHE[0]


def _shard(x, wq, bq, wk, bk, wv, bv, wo):
    in_maps = []
    for c in range(NCORES):
        b, hg = c // 4, c % 4
        I = slice(256 * hg, 256 * hg + 256)
        m = {
            "xT": np.ascontiguousarray(x[b].T),
            "wqT": np.ascontiguousarray(wq[I, :].T),
            "wkT": np.ascontiguousarray(wk[I, :].T),
            "wvT": np.ascontiguousarray(wv[I, :].T),
            "bq2": np.ascontiguousarray(bq[I].reshape(MO, P).T),
            "bk2": np.ascontiguousarray(bk[I].reshape(MO, P).T),
            "bv2": np.ascontiguousarray(np.broadcast_to(bv[I], (P, 256))),
            "woT": np.ascontiguousarray(wo[:, I].T),
            "ones_d": np.ones((P, 64), np.float32),
            "vones_d": np.ones((P, NKT, HEADS, 1), np.float32),
        }
        in_maps.append({k: v.astype(np.float32, copy=False) for k, v in m.items()})
    return in_maps


def kernel(x, wq, bq, wk, bk, wv, bv, wo, bo):
    x = np.asarray(x, dtype=np.float32)
    nc = _get_nc()
    in_maps = _shard(x, np.asarray(wq), np.asarray(bq), np.asarray(wk),
                     np.asarray(bk), np.asarray(wv), np.asarray(bv),
                     np.asarray(wo))
    res = run_bass_kernel_spmd(nc, in_maps, list(range(NCORES)), trace=TRACE)
    LAST_RESULT.clear()
    LAST_RESULT["exec_time_ns"] = res.exec_time_ns
    LAST_RESULT["mean_exec_time_ns"] = res.mean_exec_time_ns

    out = np.zeros((B, S, H), dtype=np.float64)
    for c in range(NCORES):
        out[c // 4] += res.results[c]["poutT"].T
    out += np.asarray(bo, dtype=np.float64)
    return out.astype(np.float32)



# revision 12
# speedup vs baseline: 1.0195x; 1.0195x over previous
"""8-way sharded MultiHeadAttention for Trainium2 (Bass/Tile).

Problem: B=2, S=2048, H=1024, NH=16 heads of D=64.
  out = softmax((x@wq.T+bq) @ (x@wk.T+bk).T / sqrt(D)) @ (x@wv.T+bv),
  concat heads, @ wo.T + bo.

Sharding (Megatron-style tensor parallel over 8 NeuronCores):
  core c owns batch b = c//4 and the 4 heads 4*(c%4)..4*(c%4)+3
  (feature columns Ic = 256*(c%4) .. +256 of q/k/v).
  - column-parallel QKV projections, attention fully local per head,
  - row-parallel output projection producing a partial [H, S] result;
    the 4 partials per batch are summed on the host.

v2 design notes (vs the fp32r baseline):
  - All matmul operands are fp16: same 1 cyc/row PE rate as fp32r, but
    half the DMA/SBUF footprint and 2x faster weight loads (FWL).
    PSUM accumulation stays fp32.
  - The ACT engine's exp stream (128 exps of [128,1024], ~134us busy)
    and the PE matmul stream (~150us) are the two long poles; the
    schedule overlaps them:
      * x streams in k-chunks over 4 DGE queues; the m=0 (heads 0/1)
        q/k projections contract each chunk as it lands into 4
        concurrent PSUM accumulators (scoped 8-bank pool), so the
        first scores + exp issue right after the last chunk.
      * a dummy exp at t=0 preloads the ACT exp table set (~2.7us).
      * the v projection trickles through the shared PSUM ring one
        key-chunk ahead of the attn@v that consumes it.
      * the m=1 (heads 2/3) q/k projections and the k2=0 halves of the
        out-projection run as background chunks (transient PSUM +
        fp16 SBUF accumulation) in the PE slack of later phases.
  - scores are computed transposed ([key, query]); the softmax sum
    folds into attn@v via a ones-augmented V ([v | 1]).  exp runs on
    the scalar engine straight out of PSUM with the 1/sqrt(D) scale
    fused.  No max-subtraction (|scores/8| < ~5.5, exp safe in fp32).
  - the head pair sits at prow 0/64, so the two K=64 score matmuls
    land in disjoint PE row groups (tile_position) and overlap on HW.
  - softmax normalization: 1/Z (DVE reciprocal) is broadcast across
    partitions by GPSIMD partition_broadcast (off the PE) and fused
    into the PSUM eviction multiply.
  - the out-projection's k2=1 pass fuses the add of the saved k2=0
    partial into the PSUM eviction, and the partial output is written
    back fp16, so the tail after the last exp is only ~5us.
"""

import sys

for _p in ("/opt/trn_rl_repo", "/root/.axon_site/_ro/trn_rl_repo"):
    if _p not in sys.path:
        sys.path.append(_p)

from contextlib import ExitStack

import numpy as np

import concourse.bass as bass
import concourse.mybir as mybir
import concourse.tile as tile
from concourse import bacc
from concourse.bass_utils import run_bass_kernel_spmd

F32 = mybir.dt.float32
F16 = mybir.dt.float16
AF = mybir.ActivationFunctionType
ADD = mybir.AluOpType.add
MULT = mybir.AluOpType.mult

P = 128
B = 2
S = 2048          # tokens
H = 1024          # hidden
KO = H // P       # 8 k-chunks for the QKV projections
MO = 2            # 256 local features / 128
HEADS = 4         # heads per core
D = 64
NKT = S // P      # 16 key chunks
HALF = 1024       # query half width
NCORES = 8
VPAD = 72         # per-(kt, head) v row stride in elems (65 used, 16B-aligned)
VLEAD = 3         # v-projection chunks emitted ahead of the attn@v consumer

# test.py can flip these before calling kernel()
TRACE = False
DEBUG = False
LAST_RESULT = {}


def build_mha_kernel(nc: bass.Bass):
    xT = nc.declare_dram_parameter("xT", [H, S], F16, isOutput=False)
    wqT = nc.declare_dram_parameter("wqT", [H, 256], F16, isOutput=False)
    wkT = nc.declare_dram_parameter("wkT", [H, 256], F16, isOutput=False)
    wvT = nc.declare_dram_parameter("wvT", [H, 256], F16, isOutput=False)
    bq2 = nc.declare_dram_parameter("bq2", [P, MO], F32, isOutput=False)
    bk2 = nc.declare_dram_parameter("bk2", [P, MO], F32, isOutput=False)
    bv2 = nc.declare_dram_parameter("bv2", [P, 256], F32, isOutput=False)
    woT = nc.declare_dram_parameter("woT", [256, H], F16, isOutput=False)
    vones_d = nc.declare_dram_parameter("vones_d", [P, NKT, HEADS, 1], F16,
                                        isOutput=False)
    ones_d = nc.declare_dram_parameter("ones_d", [P, 64], mybir.dt.float32r,
                                       isOutput=False)
    poutT = nc.declare_dram_parameter("poutT", [H, S], F16, isOutput=True)
    if DEBUG:
        dbg_q = nc.declare_dram_parameter("dbg_q", [P, MO, S], F16,
                                          isOutput=True)
        dbg_k = nc.declare_dram_parameter("dbg_k", [P, MO, S], F16,
                                          isOutput=True)
        dbg_v = nc.declare_dram_parameter("dbg_v", [P, NKT, HEADS, VPAD], F16,
                                          isOutput=True)
        dbg_c = nc.declare_dram_parameter("dbg_c", [P, MO, S], F16,
                                          isOutput=True)

    xT_r = xT.rearrange("(o p) n -> p o n", p=P)        # [128, 8, 2048]
    wq_r = wqT.rearrange("(o p) m -> p o m", p=P)       # [128, 8, 256]
    wk_r = wkT.rearrange("(o p) m -> p o m", p=P)
    wv_r = wvT.rearrange("(o p) m -> p o m", p=P)
    wo_r = woT.rearrange("(o p) m -> p o m", p=P)       # [128, 2, 1024]
    pout_r = poutT.rearrange("(o p) n -> p o n", p=P)   # [128, 8, 2048]

    with tile.TileContext(nc) as tc, ExitStack() as ctx:
        xp = ctx.enter_context(tc.tile_pool(name="xp", bufs=1))
        wp = ctx.enter_context(tc.tile_pool(name="wp", bufs=1))
        qk = ctx.enter_context(tc.tile_pool(name="qk", bufs=1))
        vp = ctx.enter_context(tc.tile_pool(name="vp", bufs=1))
        cx = ctx.enter_context(tc.tile_pool(name="cx", bufs=1))
        pp = ctx.enter_context(tc.tile_pool(name="pp", bufs=4))
        nm = ctx.enter_context(tc.tile_pool(name="nm", bufs=2))
        ob = ctx.enter_context(tc.tile_pool(name="ob", bufs=4))
        oa = ctx.enter_context(tc.tile_pool(name="oa", bufs=1))

        x_sb = xp.tile([P, KO, S], F16)
        wq_sb = wp.tile([P, KO, 256], F16, tag="wq")
        wk_sb = wp.tile([P, KO, 256], F16, tag="wk")
        wv_sb = wp.tile([P, KO, 256], F16, tag="wv")
        wo_sb = wp.tile([P, MO, H], F16, tag="wo")
        bq_sb = wp.tile([P, MO], F32, tag="bq")
        bk_sb = wp.tile([P, MO], F32, tag="bk")
        bv_sb = wp.tile([P, 256], F32, tag="bv")

        qT_sb = qk.tile([P, MO, S], F16, tag="q")       # [feat, token]
        kT_sb = qk.tile([P, MO, S], F16, tag="k")
        v_sb = vp.tile([P, NKT, HEADS, VPAD], F16)      # [tok, kt, h, v|1|pad]
        ctx_sb = cx.tile([P, MO, S], F16)
        # k2=0 partials of the two out-projection halves
        oacc = [oa.tile([P, KO, HALF], F16, tag=f"oacc{t}", name=f"oacc{t}")
                for t in range(2)]

        # ---- ACT exp table preload: one dummy exp at t=0 hides the
        # ~2.7us table-set load before the first real scores arrive
        dum = nm.tile([P, 8], F32, tag="dummy")
        nc.vector.memset(dum[0:1, :], 0.0)
        nc.scalar.activation(dum[0:1, :], dum[0:1, :], AF.Exp, scale=1.0)

        # ---- input DMA over 4 DGE queues: x chunks + the m=0 q/k weight
        # halves (they gate the first exp) + wv (gates the v trickle)
        # first; m=1 halves, wo and the ones column behind them.
        qs = [nc.sync, nc.gpsimd, nc.scalar]
        for k in range(KO):
            qs[k % 3].dma_start(x_sb[:, k, :], xT_r[:, k, :])
            qs[(k + 1) % 3].dma_start(wq_sb[:, k, 0:P], wq_r[:, k, 0:P])
            qs[(k + 2) % 3].dma_start(wk_sb[:, k, 0:P], wk_r[:, k, 0:P])
            qs[k % 3].dma_start(wv_sb[:, k, :], wv_r[:, k, :])
        nc.sync.dma_start(bq_sb[:], bq2[:])
        nc.gpsimd.dma_start(bk_sb[:], bk2[:])
        nc.scalar.dma_start(bv_sb[:], bv2[:])
        nc.scalar.dma_start(v_sb[:, :, :, 64:65], vones_d[:])
        ones_sb = wp.tile([P, 64], mybir.dt.float32r, tag="ones")
        nc.sync.dma_start(ones_sb[:], ones_d[:])
        for k in range(KO):
            qs[k % 3].dma_start(wq_sb[:, k, P:256], wq_r[:, k, P:256])
            qs[(k + 1) % 3].dma_start(wk_sb[:, k, P:256], wk_r[:, k, P:256])
        for k2 in range(MO):
            nc.gpsimd.dma_start(wo_sb[:, k2, :], wo_r[:, k2, :])

        # ---- m=0 (heads 0/1) q/k projections, k-outer: all four
        # accumulators (proj x token-half) live at once in a scoped
        # 8-bank PSUM pool so each x chunk is contracted the moment it
        # lands; the first scores issue right after the last chunk.
        with tc.tile_pool(name="ldp", bufs=4, space="PSUM") as ldp:
            ld = {}
            for pi in range(2):
                for tp2 in range(2):
                    ld[(pi, tp2)] = ldp.tile([P, HALF], F32, tag="ld",
                                             name=f"ld{pi}{tp2}")
            for k in range(KO):
                for pi in range(2):
                    w_sb = wq_sb if pi == 0 else wk_sb
                    for tp2 in range(2):
                        for j in range(2):
                            nc.tensor.matmul(
                                ld[(pi, tp2)][:, j * 512:(j + 1) * 512],
                                lhsT=w_sb[:, k, 0:P],
                                rhs=x_sb[:, k, tp2 * HALF + j * 512:
                                         tp2 * HALF + (j + 1) * 512],
                                start=(k == 0), stop=(k == KO - 1),
                            )
            with nc.allow_low_precision(reason="fp16 q/k store, fp32 accum"):
                for tp2 in range(2):
                    # k-proj evict on ACT (idle here), q-proj on DVE: the
                    # (k,tp0)+(q,tp0) pair gates the first scores
                    nc.scalar.activation(
                        kT_sb[:, 0, tp2 * HALF:(tp2 + 1) * HALF],
                        ld[(1, tp2)][:], AF.Identity, bias=bk_sb[:, 0:1])
                    nc.vector.tensor_tensor(
                        qT_sb[:, 0, tp2 * HALF:(tp2 + 1) * HALF],
                        ld[(0, tp2)][:],
                        bq_sb[:, 0:1].to_broadcast((P, HALF)),
                        ADD,
                    )

        big = ctx.enter_context(tc.tile_pool(name="big", bufs=2, space="PSUM"))
        cxp = ctx.enter_context(tc.tile_pool(name="cxp", bufs=2, space="PSUM"))

        # ---- background chunk emitters ----

        def v_chunk(kt):
            # v projection for one key-chunk, all 4 heads ([tok, feat])
            def emit():
                ps = big.tile([P, 256], F32, tag="big", name="vps")
                for k in range(KO):
                    nc.tensor.matmul(
                        ps[:],
                        lhsT=x_sb[:, k, kt * P:(kt + 1) * P],
                        rhs=wv_sb[:, k, :],
                        start=(k == 0), stop=(k == KO - 1),
                    )
                with nc.allow_low_precision(reason="fp16 v store"):
                    nc.vector.tensor_tensor(
                        v_sb[:, kt, :, 0:64], ps[:], bv_sb[:], ADD)
            return emit

        def m1qk_chunks():
            # m=1 (heads 2/3) q/k projections: one 8-matmul transient-PSUM
            # chunk per (proj, token-half, 512-block), evicted with a single
            # per-partition bias add
            cl = []
            for pi in range(2):
                for tp2 in range(2):
                    for j in range(2):
                        def emit(pi=pi, tp2=tp2, j=j):
                            w_sb = wq_sb if pi == 0 else wk_sb
                            dst = qT_sb if pi == 0 else kT_sb
                            b_sb = bq_sb if pi == 0 else bk_sb
                            lo = tp2 * HALF + j * 512
                            ps = big.tile([P, 512], F32, tag="big",
                                          name="m1")
                            for kk in range(KO):
                                nc.tensor.matmul(
                                    ps[:],
                                    lhsT=w_sb[:, kk, P:256],
                                    rhs=x_sb[:, kk, lo:lo + 512],
                                    start=(kk == 0),
                                    stop=(kk == KO - 1),
                                )
                            with nc.allow_low_precision(
                                    reason="fp16 q/k store"):
                                nc.vector.tensor_scalar_add(
                                    dst[:, 1, lo:lo + 512], ps[:],
                                    b_sb[:, 1:2])
                        cl.append(emit)
            return cl

        def op_passA_chunks(tp):
            # out-projection k2=0 half -> fp16 SBUF partial
            cl = []
            for m in range(KO):
                def emit(m=m):
                    ps = big.tile([P, HALF], F32, tag="big", name="opA")
                    for j in range(2):
                        nc.tensor.matmul(
                            ps[:, j * 512:(j + 1) * 512],
                            lhsT=wo_sb[:, 0, m * P:(m + 1) * P],
                            rhs=ctx_sb[:, 0, tp * HALF + j * 512:
                                       tp * HALF + (j + 1) * 512],
                            start=True, stop=True,
                        )
                    with nc.allow_low_precision(reason="fp16 partial out"):
                        nc.vector.tensor_copy(oacc[tp][:, m, :], ps[:])
                cl.append(emit)
            return cl

        def op_passB_chunks(tp):
            # out-projection k2=1 half; the saved k2=0 partial is added
            # during the PSUM eviction, then DMA out (fp16)
            cl = []
            for m in range(KO):
                def emit(m=m):
                    lo = tp * HALF
                    ps = big.tile([P, HALF], F32, tag="big", name="opB")
                    for j in range(2):
                        nc.tensor.matmul(
                            ps[:, j * 512:(j + 1) * 512],
                            lhsT=wo_sb[:, 1, m * P:(m + 1) * P],
                            rhs=ctx_sb[:, 1, lo + j * 512:lo + (j + 1) * 512],
                            start=True, stop=True,
                        )
                    ot = ob.tile([P, HALF], F16, tag="ot", name="ot")
                    with nc.allow_low_precision(reason="fp16 partial out"):
                        nc.vector.tensor_tensor(
                            ot[:], ps[:], oacc[tp][:, m, :], ADD)
                    nc.sync.dma_start(pout_r[:, m, lo:lo + HALF], ot[:])
                cl.append(emit)
            return cl

        # ---- softmax-normalize and evict one head-half from PSUM.
        # 1/Z straight from the PSUM Z row and raw ctx copied to SBUF (two
        # independent DVE ops) release the ctx PSUM slot fast; the
        # partition broadcast of 1/Z runs on the idle GPSIMD engine.
        def normalize(half, h, ctx_ps):
            o, prow = h // 2, 64 * (h % 2)
            rst = nm.tile([P, HALF], mybir.dt.float32r, tag="rst", name="rst")
            with nc.allow_low_precision(reason="1/Z in f32r, Z ~ O(S)"):
                nc.vector.reciprocal(rst[64:65, :], ctx_ps[64:65, :])
            traw = nm.tile([P, HALF], F32, tag="traw", name="traw")
            nc.vector.tensor_copy(traw[0:64, :], ctx_ps[0:64, :])
            bc = big.tile([P, HALF], F32, tag="big", name="bc")
            for j in range(2):
                nc.tensor.matmul(
                    bc[0:64, j * 512:(j + 1) * 512],
                    lhsT=ones_sb[64:65, :],
                    rhs=rst[64:65, j * 512:(j + 1) * 512],
                    start=True, stop=True,
                )
            with nc.allow_low_precision(reason="fp16 ctx store"):
                nc.vector.tensor_tensor(
                    ctx_sb[prow:prow + 64, o, half * HALF:(half + 1) * HALF],
                    traw[0:64, :],
                    bc[0:64, :],
                    MULT,
                )

        # ---- attention for one (query-half, head-pair); two heads
        # interleaved per key-chunk (they sit at prow 0/64, so their K=64
        # score matmuls land in disjoint PE row groups and overlap on HW).
        # bg: background emitters, drained at `rate` per key-chunk.
        def emit_attn(half, pair, bg, rate=1):
            ctx_t = [None, None]
            carry = 0.0
            for kt in range(NKT):
                for hi in range(2):
                    h = 2 * pair + hi
                    o, prow = h // 2, 64 * (h % 2)
                    qh = qT_sb[prow:prow + 64, o, :]
                    kh = kT_sb[prow:prow + 64, o, :]
                    sp = big.tile([P, HALF], F32, tag="big", name="sp")
                    for j in range(2):
                        nc.tensor.matmul(
                            sp[:, j * 512:(j + 1) * 512],
                            lhsT=kh[:, kt * P:(kt + 1) * P],
                            rhs=qh[:, half * HALF + j * 512:
                                   half * HALF + (j + 1) * 512],
                            start=True, stop=True,
                        )
                    pt = pp.tile([P, HALF], F16, tag="pt", name="pt")
                    nc.scalar.activation(pt[:], sp[:], AF.Exp, scale=0.125)
                    if kt == 0:
                        ctx_t[hi] = cxp.tile([P, HALF], F32, tag="ctx",
                                             name="ctx")
                    for j in range(2):
                        nc.tensor.matmul(
                            ctx_t[hi][0:65, j * 512:(j + 1) * 512],
                            lhsT=v_sb[:, kt, h, 0:65],
                            rhs=pt[:, j * 512:(j + 1) * 512],
                            start=(kt == 0), stop=(kt == NKT - 1),
                        )
                carry += rate
                while bg and carry >= 1.0:
                    bg.pop(0)()
                    carry -= 1.0
            for hi in range(2):
                normalize(half, 2 * pair + hi, ctx_t[hi])

        # ---- phase schedule.  Heads 0/1 run both query halves first so
        # the m=1 projections have 2 phases of PE slack before heads 2/3
        # need them.  v trickles VLEAD chunks ahead of its consumer.
        vq = [v_chunk(kt) for kt in range(NKT)]
        for kt in range(VLEAD):
            vq.pop(0)()
        emit_attn(0, 0, vq)                      # heads 0/1, q 0..1023
        assert not vq
        emit_attn(1, 0, m1qk_chunks(), rate=1)   # heads 0/1, q 1024..2047
        emit_attn(0, 1, op_passA_chunks(0),      # heads 2/3, q 0..1023
                  rate=0.5)
        opb0 = op_passA_chunks(1) + op_passB_chunks(0)
        emit_attn(1, 1, opb0, rate=1)            # heads 2/3, q 1024..2047
        for e in opb0:
            e()
        for e in op_passB_chunks(1):             # tail: ~5us
            e()
        if DEBUG:
            nc.sync.dma_start(dbg_q[:], qT_sb[:])
            nc.sync.dma_start(dbg_k[:], kT_sb[:])
            nc.sync.dma_start(dbg_v[:], v_sb[:])
            nc.sync.dma_start(dbg_c[:], ctx_sb[:])

    return nc


_NC_CACHE = []


def _get_nc():
    if not _NC_CACHE:
        nc = bacc.Bacc(
            "TRN2",
            target_bir_lowering=False,
            debug=False,
            enable_asserts=False,
            num_devices=NCORES,
        )
        build_mha_kernel(nc)
        nc.finalize()
        _NC_CACHE.append(nc)
    return _NC_CACHE[0]


def _shard(x, wq, bq, wk, bk, wv, bv, wo):
    in_maps = []
    f16 = np.float16
    for c in range(NCORES):
        b, hg = c // 4, c % 4
        I = slice(256 * hg, 256 * hg + 256)
        m = {
            "xT": np.ascontiguousarray(x[b].T).astype(f16),
            "wqT": np.ascontiguousarray(wq[I, :].T).astype(f16),
            "wkT": np.ascontiguousarray(wk[I, :].T).astype(f16),
            "wvT": np.ascontiguousarray(wv[I, :].T).astype(f16),
            "bq2": np.ascontiguousarray(
                bq[I].reshape(MO, P).T).astype(np.float32),
            "bk2": np.ascontiguousarray(
                bk[I].reshape(MO, P).T).astype(np.float32),
            "bv2": np.ascontiguousarray(
                np.broadcast_to(bv[I], (P, 256))).astype(np.float32),
            "woT": np.ascontiguousarray(wo[:, I].T).astype(f16),
            "vones_d": np.ones((P, NKT, HEADS, 1), f16),
            "ones_d": np.ones((P, 64), np.float32),
        }
        in_maps.append(m)
    return in_maps


def kernel(x, wq, bq, wk, bk, wv, bv, wo, bo):
    x = np.asarray(x, dtype=np.float32)
    nc = _get_nc()
    in_maps = _shard(x, np.asarray(wq), np.asarray(bq), np.asarray(wk),
                     np.asarray(bk), np.asarray(wv), np.asarray(bv),
                     np.asarray(wo))
    res = run_bass_kernel_spmd(nc, in_maps, list(range(NCORES)), trace=TRACE)
    LAST_RESULT.clear()
    LAST_RESULT["exec_time_ns"] = res.exec_time_ns
    LAST_RESULT["mean_exec_time_ns"] = getattr(res, "mean_exec_time_ns", None)

    out = np.zeros((B, S, H), dtype=np.float64)
    for c in range(NCORES):
        out[c // 4] += res.results[c]["poutT"].astype(np.float32).T
    out += np.asarray(bo, dtype=np.float64)
    return out.astype(np.float32)
